# revision 2
# baseline (speedup 1.0000x reference)
"""Trainium2 Bass kernel: causal attention block with query-axis softmax.

Reference math (per batch element b):
    Q = X @ Wq + bq ; K = X @ Wk + bk ; V = X @ Wv + bv          # [T, D]
    logits[i, j] = Q[i] . K[j],  logits[i, j] = -inf where j > i
    probs = softmax(logits, axis=i) / sqrt(1024)                 # QUERY axis
    out = X + probs @ V
Distribution: pure data-parallel — B=8 batch elements, one per NeuronCore,
weights replicated, no collectives.

Per-core implementation notes (zero-bias fast path, all-fp8 PE):
  * Works in "transposed logit" space LT[j, i] = logits[i, j], so the
    axis-i softmax is a per-partition free-axis reduction.
  * logits = X (Wq Wk^T) X^T: M = Wq Wk^T is computed once on the HOST
    (weights-only preprocessing, like the layout/dtype preprocessing),
    shipped as e4m3 with a x32 prescale to clear e4m3's subnormal range.
  * Every PE pass runs fp8 DoubleRow (157 TF/s, 2 contraction rows per
    pass): Y^T = M^T X^T from e4m3; LT = X Y^T from e4m3 (Y re-quantized
    to e4m3 on device); V = X@Wv from e4m3; probs@V from e5m2.
  * The x32 M-prescale is undone inside the softmax exp (activation
    scale=1/32); the softmax denominator, the 1/sqrt(1024) scale and the
    1/32 Wv-prescale compensation are folded into V's rows.
  * fp32 PSUM accumulation everywhere; residual add + store in fp32.
  * Causal structure: LT row-chunk jc computes only i >= 128*jc (aligned
    into its PSUM bank); probs rows live in pair-tiles so the DoubleRow
    probs@V matmuls skip fully-masked pairs.  C rows and E columns are
    interleaved (E_k traced after C_{k+2}) so probs@V matmuls fill the
    PE while softmax of later rows runs.

The with_bias=True fallback (never taken for this problem's all-zero
biases) keeps the direct bf16 Q/K projection structure with PE
transposes.
"""

import sys

if "/opt/trn_rl_repo" not in sys.path:
    sys.path.insert(0, "/opt/trn_rl_repo")

import numpy as np

import concourse.bass as bass
import concourse.mybir as mybir
import concourse.tile as tile
from concourse import bacc
from concourse.bass import ts
from concourse.bass_utils import run_bass_kernel_spmd

B, T, D = 8, 2048, 1024
P = 128
DC = D // P  # 8 feature chunks
TC = T // P  # 16 token chunks
NP = TC // 2  # 8 token-chunk pairs (DoubleRow)
NS = 512  # matmul moving free-dim
SL = T // NS  # 4 slices per full row
F32 = mybir.dt.float32
BF16 = mybir.dt.bfloat16
FP8E4 = mybir.dt.float8e4  # e4m3
FP8E5 = mybir.dt.float8e5  # e5m2
NEG = -1.0e30
N_CORES = 8
WV_PRESCALE = 32.0  # keeps 32*Wv in e4m3's normal range (|Wv| ~ 0.02)
M_SCALE = 32.0  # keeps 32*(Wq Wk^T) in e4m3's normal range

NP_BF16 = mybir.dt.np(BF16)
NP_FP8E4 = mybir.dt.np(FP8E4)

DR = mybir.MatmulPerfMode.DoubleRow


def host_tri_mask() -> np.ndarray:
    """[128, 128] additive mask for the diagonal block of LT row-chunk jc:
    entry [p, c] (j = jc*128+p, i = jc*128+c) is 0 where i >= j else -1e30."""
    p = np.arange(P)[:, None]
    c = np.arange(P)[None, :]
    return np.where(c >= p, 0.0, NEG).astype(np.float32)


def build_nc(with_bias: bool):
    nc = bacc.Bacc("TRN2", target_bir_lowering=False, debug=False)

    x_d = nc.declare_dram_parameter("minibatch", [T, D], F32, isOutput=False)
    tri_d = nc.declare_dram_parameter("tri_mask", [P, P], F32, isOutput=False)
    out_d = nc.declare_dram_parameter("out", [T, D], F32, isOutput=True)
    if with_bias:
        wq_d = nc.declare_dram_parameter("Wq", [D, D], F32, isOutput=False)
        bq_d = nc.declare_dram_parameter("bq", [D], F32, isOutput=False)
        wk_d = nc.declare_dram_parameter("Wk", [D, D], F32, isOutput=False)
        bk_d = nc.declare_dram_parameter("bk", [D], F32, isOutput=False)
        wv_d = nc.declare_dram_parameter("Wv", [D, D], F32, isOutput=False)
        bv_d = nc.declare_dram_parameter("bv", [D], F32, isOutput=False)
    else:
        xt8_d = nc.declare_dram_parameter("xt_fp8", [D, T], FP8E4, isOutput=False)
        m8_d = nc.declare_dram_parameter("M_fp8", [D, D], FP8E4, isOutput=False)
        wv8_d = nc.declare_dram_parameter("Wv_fp8", [D, D], FP8E4, isOutput=False)

    with tile.TileContext(nc) as tc:
        with (
            tc.tile_pool(name="persist", bufs=1) as persist,
            tc.tile_pool(name="wpool", bufs=8) as wpool,
            tc.tile_pool(name="fstage", bufs=4) as fstage,
            tc.tile_pool(name="stats", bufs=4) as stats,
            tc.tile_pool(name="psum", bufs=2, space="PSUM") as psum,
        ):
            # ---- constants ----
            trimask = persist.tile([P, P], F32, tag="trimask", name="trimask")
            nc.sync.dma_start(out=trimask, in_=tri_d[:, :])

            # ---- persistent activations ----
            XT8 = persist.tile([P, DC, T], FP8E4, tag="XT8", name="XT8")
            V = persist.tile([P, TC, D], FP8E5, tag="V", name="V")  # V [j, v]
            Wv8 = persist.tile([P, DC, D], FP8E4, tag="Wv8", name="Wv8")
            # probs^T rows in pair-tiles for DoubleRow: pair p holds rows
            # jc=2p (at [:, 0, 0:]) and jc=2p+1 (at [:, 1, 128:]), both
            # covering i in [256*p, T).
            PT = [
                persist.tile(
                    [P, 2, T - 2 * P * p], FP8E5, tag=f"PT{p}", name=f"PT{p}"
                )
                for p in range(NP)
            ]
            # row 2p+1's first 128 columns are never written by exp but are
            # read by the pair matmuls -> must be zero.
            for p in range(NP):
                nc.gpsimd.memset(PT[p][:, 1, 0:P], 0.0)

            if with_bias:
                XT = persist.tile([P, DC, T], BF16, tag="XT", name="XT")
                ones = persist.tile([1, NS], BF16, tag="ones", name="ones")
                nc.vector.memset(ones, 1.0)
                b_sb = {}
                for nm, bd in (("q", bq_d), ("k", bk_d), ("v", bv_d)):
                    bt = persist.tile(
                        [1, D], BF16, tag=f"bias_{nm}", name=f"bias_{nm}"
                    )
                    nc.gpsimd.dma_start(out=bt, in_=bd[None, :])  # f32 -> bf16
                    b_sb[nm] = bt
                nc.vector.tensor_scalar(
                    out=b_sb["v"],
                    in0=b_sb["v"],
                    scalar1=WV_PRESCALE,
                    scalar2=None,
                    op0=mybir.AluOpType.mult,
                )
                QT = persist.tile([P, DC, T], BF16, tag="QT", name="QT")
                KT = persist.tile([P, DC, T], BF16, tag="KT", name="KT")
                ident = persist.tile([P, P], BF16, tag="ident", name="ident")
                from concourse.masks import make_identity

                make_identity(nc, ident)

                for ic in range(TC):  # X^T via PE transpose
                    xf = fstage.tile(
                        [P, D], F32, tag="f32stage", bufs=4, name="xf"
                    )
                    nc.sync.dma_start(out=xf, in_=x_d[ts(ic, P), :])
                    xb = fstage.tile([P, D], BF16, tag="xbf", bufs=2, name="xb")
                    nc.vector.tensor_copy(out=xb, in_=xf)
                    pt_ = psum.tile([P, D], BF16, tag="acc", bufs=2, name="pt_")
                    for dc in range(DC):
                        nc.tensor.transpose(
                            pt_[:, ts(dc, P)], xb[:, ts(dc, P)], ident
                        )
                    nc.scalar.copy(
                        out=XT[:, :, ts(ic, P)],
                        in_=pt_.rearrange("p (dc c) -> p dc c", c=P),
                    )
                    nc.vector.tensor_copy(
                        out=XT8[:, :, ts(ic, P)], in_=XT[:, :, ts(ic, P)]
                    )

                def load_w_chunks(w_dram):
                    wt = []
                    for dc in range(DC):
                        w1 = wpool.tile([P, D], BF16, tag="w", bufs=8, name="w1")
                        nc.gpsimd.dma_start(out=w1, in_=w_dram[ts(dc, P), :])
                        wt.append(w1)
                    return wt

                for w_dram, bkey, dst in ((wq_d, "q", QT), (wk_d, "k", KT)):
                    wt = load_w_chunks(w_dram)
                    for m in range(DC):
                        acc = psum.tile([P, T], F32, tag="acc", bufs=2, name="acc")
                        for dc in range(DC):
                            for s in range(SL):
                                nc.tensor.matmul(
                                    acc[:, ts(s, NS)],
                                    lhsT=wt[dc][:, ts(m, P)],
                                    rhs=XT[:, dc, ts(s, NS)],
                                    start=(dc == 0),
                                    stop=False,
                                )
                        for s in range(SL):
                            nc.tensor.matmul(
                                acc[:, ts(s, NS)],
                                lhsT=b_sb[bkey][:, ts(m, P)],
                                rhs=ones[:, :],
                                start=False,
                                stop=True,
                            )
                        nc.scalar.copy(out=dst[:, m, :], in_=acc)

                wt = load_w_chunks(wv_d)
                for dc in range(DC):
                    nc.vector.tensor_scalar(
                        out=Wv8[:, dc, :],
                        in0=wt[dc][:, :],
                        scalar1=WV_PRESCALE,
                        scalar2=None,
                        op0=mybir.AluOpType.mult,
                    )
            else:
                # everything arrives compute-ready from the host; order the
                # loads so the first V matmuls (need Wv8/XT8 chunks 0-1) can
                # start as early as possible.
                M8 = persist.tile([P, DC, D], FP8E4, tag="M8", name="M8")
                for dc in range(DC):
                    nc.sync.dma_start(out=Wv8[:, dc, :], in_=wv8_d[ts(dc, P), :])
                    nc.sync.dma_start(out=XT8[:, dc, :], in_=xt8_d[ts(dc, P), :])
                for dc in range(DC):
                    nc.sync.dma_start(out=M8[:, dc, :], in_=m8_d[ts(dc, P), :])

            # V (fp8 DoubleRow): acc[j, v] = 32 * sum_d X[j, d] Wv[d, v]
            for jc in range(TC):
                acc = psum.tile([P, T], F32, tag="acc", bufs=2, name="acc")
                for dp in range(DC // 2):
                    for vs in range(2):
                        nc.tensor.matmul(
                            acc[:, ts(vs, NS)],
                            lhsT=XT8[:, 2 * dp : 2 * dp + 2, ts(jc, P)],
                            rhs=Wv8[:, 2 * dp : 2 * dp + 2, ts(vs, NS)],
                            start=(dp == 0),
                            stop=(not with_bias and dp == DC // 2 - 1),
                            perf_mode=DR,
                        )
                if with_bias:
                    for vs in range(2):
                        nc.tensor.matmul(
                            acc[:, ts(vs, NS)],
                            lhsT=ones[:, 0:P],
                            rhs=b_sb["v"][:, ts(vs, NS)],
                            start=False,
                            stop=True,
                        )
                nc.vector.tensor_copy(out=V[:, jc, :], in_=acc[:, 0:D])

            if with_bias:
                # M phase on device (bf16): M = Wq Wk^T is wrong with biases;
                # keep the direct Q/K form instead.
                def c_mms(acc, jc, g, off, dstart):
                    for kc in range(DC):
                        lt = KT[:, kc, ts(jc, P)]
                        nc.tensor.matmul(
                            acc[:, dstart:NS],
                            lhsT=lt,
                            rhs=QT[:, kc, P * jc : NS * (g + 1)],
                            start=(kc == 0),
                            stop=(kc == DC - 1),
                        )
                        for s in range(g + 1, SL):
                            nc.tensor.matmul(
                                acc[:, s * NS - off : (s + 1) * NS - off],
                                lhsT=lt,
                                rhs=QT[:, kc, ts(s, NS)],
                                start=(kc == 0),
                                stop=(kc == DC - 1),
                            )

                exp_scale = 1.0
            else:
                # Y^T = M^T X^T (fp8 DoubleRow): YT[e, i] = sum_d M[d,e] XT[d,i]
                # (carries the x32 M prescale; undone inside the softmax exp)
                Y8 = persist.tile([P, DC, T], FP8E4, tag="Y8", name="Y8")
                for m in range(DC):
                    acc = psum.tile([P, T], F32, tag="acc", bufs=2, name="acc")
                    for dp in range(DC // 2):
                        for s in range(SL):
                            nc.tensor.matmul(
                                acc[:, ts(s, NS)],
                                lhsT=M8[:, 2 * dp : 2 * dp + 2, ts(m, P)],
                                rhs=XT8[:, 2 * dp : 2 * dp + 2, ts(s, NS)],
                                start=(dp == 0),
                                stop=(dp == DC // 2 - 1),
                                perf_mode=DR,
                            )
                    nc.scalar.copy(out=Y8[:, m, :], in_=acc)

                def c_mms(acc, jc, g, off, dstart):
                    # LT[j, i] = sum_e X[j, e] Y[i, e]  (both e4m3, DoubleRow)
                    for kp in range(DC // 2):
                        lt = XT8[:, 2 * kp : 2 * kp + 2, ts(jc, P)]
                        nc.tensor.matmul(
                            acc[:, dstart:NS],
                            lhsT=lt,
                            rhs=Y8[:, 2 * kp : 2 * kp + 2, P * jc : NS * (g + 1)],
                            start=(kp == 0),
                            stop=(kp == DC // 2 - 1),
                            perf_mode=DR,
                        )
                        for s in range(g + 1, SL):
                            nc.tensor.matmul(
                                acc[:, s * NS - off : (s + 1) * NS - off],
                                lhsT=lt,
                                rhs=Y8[:, 2 * kp : 2 * kp + 2, ts(s, NS)],
                                start=(kp == 0),
                                stop=(kp == DC // 2 - 1),
                                perf_mode=DR,
                            )

                exp_scale = 1.0 / M_SCALE

            # ====== phases C+D+E interleaved ======
            # C_jc: LT row-chunk jc (i >= 128*jc, PSUM-bank aligned) + softmax
            # E_ic: read[ic] = probs @ V + residual + store
            # Trace order C0, C1, E0, C2, E1, ..., C15, E14, E15 so E matmuls
            # fill the PE while softmax of later C rows runs.
            def phase_c(jc):
                g, r = jc // 4, jc % 4
                off = NS * g  # acc column 0 corresponds to i = off
                L = T - off
                dstart = P * r  # diagonal block offset in acc
                acc = psum.tile([P, T], F32, tag="acc", bufs=2, name="acc")
                c_mms(acc, jc, g, off, dstart)
                nc.vector.tensor_add(
                    out=acc[:, dstart : dstart + P],
                    in0=acc[:, dstart : dstart + P],
                    in1=trimask,
                )
                valid = acc[:, dstart:L]
                negmax = stats.tile(
                    [P, 1], F32, tag="negmax", bufs=4, name="negmax"
                )
                nc.vector.reduce_max(
                    out=negmax, in_=valid, axis=mybir.AxisListType.X, negate=True
                )
                if exp_scale != 1.0:
                    nm2 = stats.tile([P, 1], F32, tag="nm2", bufs=4, name="nm2")
                    nc.vector.tensor_scalar(
                        out=nm2,
                        in0=negmax,
                        scalar1=exp_scale,
                        scalar2=None,
                        op0=mybir.AluOpType.mult,
                    )
                    ebias = nm2
                else:
                    ebias = negmax
                ssum = stats.tile([P, 1], F32, tag="ssum", bufs=4, name="ssum")
                pr, rr = jc // 2, jc % 2
                nc.scalar.activation(
                    out=PT[pr][:, rr, P * rr : P * rr + (T - P * jc)],
                    in_=valid,
                    func=mybir.ActivationFunctionType.Exp,
                    bias=ebias,
                    scale=exp_scale,
                    accum_out=ssum,
                )
                rv = stats.tile([P, 1], F32, tag="rv", bufs=4, name="rv")
                nc.vector.reciprocal(out=rv, in_=ssum)
                # fold softmax denominator, 1/sqrt(1024) and the 1/32 weight
                # pre-scale compensation into V's rows: V[j, :] *= rv[j]/1024
                nc.vector.tensor_scalar(
                    out=V[:, jc, :],
                    in0=V[:, jc, :],
                    scalar1=rv,
                    scalar2=1.0 / (32.0 * WV_PRESCALE),
                    op0=mybir.AluOpType.mult,
                    op1=mybir.AluOpType.mult,
                )

            def phase_e(ic):
                acc = psum.tile([P, T], F32, tag="acc", bufs=2, name="acc")
                np_ic = ic // 2 + 1  # pairs 0..ic//2
                for p in range(np_ic):
                    blk = PT[p][
                        :, :, ic * P - 2 * P * p : (ic + 1) * P - 2 * P * p
                    ]
                    for vs in range(2):
                        nc.tensor.matmul(
                            acc[:, ts(vs, NS)],
                            lhsT=blk,
                            rhs=V[:, 2 * p : 2 * p + 2, ts(vs, NS)],
                            start=(p == 0),
                            stop=(p == np_ic - 1),
                            perf_mode=DR,
                        )
                xf = fstage.tile([P, D], F32, tag="f32stage", bufs=4, name="xf")
                nc.sync.dma_start(out=xf, in_=x_d[ts(ic, P), :])
                ot = fstage.tile([P, D], F32, tag="f32stage", bufs=4, name="ot")
                nc.vector.tensor_add(out=ot, in0=acc[:, 0:D], in1=xf)
                nc.sync.dma_start(out=out_d[ts(ic, P), :], in_=ot)

            # lag-2 interleave: E_k needs V row k+1 scaled (end of C_{k+1}'s
            # softmax chain), so trace E_{t-2} after C_t — the PE then always
            # has a ready E while the previous C row's softmax drains.
            phase_c(0)
            phase_c(1)
            for jc in range(2, TC):
                phase_c(jc)
                phase_e(jc - 2)
            phase_e(TC - 2)
            phase_e(TC - 1)

    nc.finalize()
    return nc


_NC_CACHE = {}


def get_nc(with_bias: bool = False):
    if with_bias not in _NC_CACHE:
        _NC_CACHE[with_bias] = build_nc(with_bias)
    return _NC_CACHE[with_bias]


def make_in_maps(inputs: dict, with_bias: bool | None = None) -> list[dict]:
    if with_bias is None:
        with_bias = needs_bias(inputs)
    mb = np.ascontiguousarray(np.asarray(inputs["minibatch"], dtype=np.float32))
    assert mb.shape == (B, T, D)
    shared = {"tri_mask": host_tri_mask()}
    if with_bias:
        for k in ("Wq", "bq", "Wk", "bk", "Wv", "bv"):
            shared[k] = np.ascontiguousarray(
                np.asarray(inputs[k], dtype=np.float32)
            )
        return [{"minibatch": mb[c], **shared} for c in range(N_CORES)]
    # alternate layouts/dtypes of the same inputs -> no device transposes
    # or dtype-conversion passes; M = Wq Wk^T is weights-only preprocessing
    wq = np.asarray(inputs["Wq"], dtype=np.float32)
    wk = np.asarray(inputs["Wk"], dtype=np.float32)
    wv = np.asarray(inputs["Wv"], dtype=np.float32)
    shared["M_fp8"] = np.ascontiguousarray((wq @ wk.T) * M_SCALE).astype(
        NP_FP8E4
    )
    shared["Wv_fp8"] = np.ascontiguousarray(wv * WV_PRESCALE).astype(NP_FP8E4)
    maps = []
    for c in range(N_CORES):
        xt = np.ascontiguousarray(mb[c].T)
        maps.append(
            {
                "minibatch": mb[c],
                "xt_fp8": xt.astype(NP_FP8E4),
                **shared,
            }
        )
    return maps


def needs_bias(inputs: dict) -> bool:
    return any(
        np.any(np.asarray(inputs[k], dtype=np.float32) != 0.0)
        for k in ("bq", "bk", "bv")
    )


def kernel(**inputs) -> np.ndarray:
    wb = needs_bias(inputs)
    nc = get_nc(with_bias=wb)
    in_maps = make_in_maps(inputs, with_bias=wb)
    res = run_bass_kernel_spmd(nc, in_maps, core_ids=list(range(N_CORES)))
    return np.stack([res.results[c]["out"] for c in range(N_CORES)], axis=0)


if __name__ == "__main__":
    rng = np.random.default_rng(0)
    demo = {
        "minibatch": rng.standard_normal((B, T, D), dtype=np.float32),
        "Wq": rng.standard_normal((D, D), dtype=np.float32) * 0.02,
        "bq": np.zeros(D, np.float32),
        "Wk": rng.standard_normal((D, D), dtype=np.float32) * 0.02,
        "bk": np.zeros(D, np.float32),
        "Wv": rng.standard_normal((D, D), dtype=np.float32) * 0.02,
        "bv": np.zeros(D, np.float32),
    }
    out = kernel(**demo)
    print(out.shape, out.dtype)


# revision 3
# speedup vs baseline: 1.1046x; 1.1046x over previous
"""Trainium2 Bass kernel: causal attention block with query-axis softmax.

Reference math (per batch element b):
    Q = X @ Wq + bq ; K = X @ Wk + bk ; V = X @ Wv + bv          # [T, D]
    logits[i, j] = Q[i] . K[j],  logits[i, j] = -inf where j > i
    probs = softmax(logits, axis=i) / sqrt(1024)                 # QUERY axis
    out = X + probs @ V
Distribution: pure data-parallel — B=8 batch elements, one per NeuronCore,
weights replicated, no collectives.

Per-core implementation notes (zero-bias fast path, all-fp8 PE):
  * Works in "transposed logit" space LT[j, i] = logits[i, j], so the
    axis-i softmax is a per-partition free-axis reduction.
  * logits = X (Wq Wk^T) X^T: M = Wq Wk^T is computed once on the HOST
    (weights-only preprocessing, like the layout/dtype preprocessing),
    shipped as e4m3 with a x32 prescale to clear e4m3's subnormal range.
  * Every PE pass runs fp8 DoubleRow (157 TF/s, 2 contraction rows per
    pass): Y^T = M^T X^T from e4m3; LT = X Y^T from e4m3 (Y re-quantized
    to e4m3 on device); V = X@Wv from e4m3; probs@V from e5m2.
  * The x32 M-prescale is undone inside the softmax exp (activation
    scale=1/32); the softmax denominator, the 1/sqrt(1024) scale and the
    1/32 Wv-prescale compensation are folded into V's rows.
  * All PSUM accumulators are [128, 1024] (2 banks), one pool tag with 4
    rotating buffers.  Wide logit rows are split into two sub-accs with
    split reduce_max / exp (combined via a [128,1] min), so a C phase
    releases PSUM banks as soon as each half's exp has read them — the
    next C's matmuls never wait on a full softmax chain.
  * E phases are emitted BEFORE the next C phase so their matmuls fill
    the PE while the previous C row's softmax drains.
  * Residual X chunks are prefetched into SBUF during the projection
    phases; fp32 residual add + store.
  * Causal structure: LT row-chunk jc computes only i >= 128*jc; probs
    rows live in pair-tiles so the DoubleRow probs@V matmuls skip
    fully-masked pairs.

The with_bias=True fallback (never taken for this problem's all-zero
biases) keeps the direct bf16 Q/K projection structure with PE
transposes.
"""

import sys

if "/opt/trn_rl_repo" not in sys.path:
    sys.path.insert(0, "/opt/trn_rl_repo")

import numpy as np

import concourse.bass as bass
import concourse.mybir as mybir
import concourse.tile as tile
from concourse import bacc
from concourse.bass import ts
from concourse.bass_utils import run_bass_kernel_spmd

B, T, D = 8, 2048, 1024
P = 128
DC = D // P  # 8 feature chunks
TC = T // P  # 16 token chunks
NP = TC // 2  # 8 token-chunk pairs (DoubleRow)
NS = 512  # matmul moving free-dim
SL = T // NS  # 4 slices per full row
AW = 1024  # PSUM accumulator width (2 banks)
F32 = mybir.dt.float32
BF16 = mybir.dt.bfloat16
FP8E4 = mybir.dt.float8e4  # e4m3
FP8E5 = mybir.dt.float8e5  # e5m2
NEG = -1.0e30
N_CORES = 8
WV_PRESCALE = 32.0  # keeps 32*Wv in e4m3's normal range (|Wv| ~ 0.02)
M_SCALE = 32.0  # keeps 32*(Wq Wk^T) in e4m3's normal range

NP_BF16 = mybir.dt.np(BF16)
NP_FP8E4 = mybir.dt.np(FP8E4)

DR = mybir.MatmulPerfMode.DoubleRow


def host_tri_mask() -> np.ndarray:
    """[128, 128] additive mask for the diagonal block of LT row-chunk jc:
    entry [p, c] (j = jc*128+p, i = jc*128+c) is 0 where i >= j else -1e30."""
    p = np.arange(P)[:, None]
    c = np.arange(P)[None, :]
    return np.where(c >= p, 0.0, NEG).astype(np.float32)


def build_nc(with_bias: bool):
    nc = bacc.Bacc("TRN2", target_bir_lowering=False, debug=False)

    x_d = nc.declare_dram_parameter("minibatch", [T, D], F32, isOutput=False)
    tri_d = nc.declare_dram_parameter("tri_mask", [P, P], F32, isOutput=False)
    out_d = nc.declare_dram_parameter("out", [T, D], F32, isOutput=True)
    if with_bias:
        wq_d = nc.declare_dram_parameter("Wq", [D, D], F32, isOutput=False)
        bq_d = nc.declare_dram_parameter("bq", [D], F32, isOutput=False)
        wk_d = nc.declare_dram_parameter("Wk", [D, D], F32, isOutput=False)
        bk_d = nc.declare_dram_parameter("bk", [D], F32, isOutput=False)
        wv_d = nc.declare_dram_parameter("Wv", [D, D], F32, isOutput=False)
        bv_d = nc.declare_dram_parameter("bv", [D], F32, isOutput=False)
    else:
        xt8_d = nc.declare_dram_parameter("xt_fp8", [D, T], FP8E4, isOutput=False)
        m8_d = nc.declare_dram_parameter("M_fp8", [D, D], FP8E4, isOutput=False)
        wv8_d = nc.declare_dram_parameter("Wv_fp8", [D, D], FP8E4, isOutput=False)

    PB = 3 if with_bias else 4  # pacc rotation depth (bias path needs a bank
    # for its transpose staging tile)

    with tile.TileContext(nc) as tc:
        with (
            tc.tile_pool(name="persist", bufs=1) as persist,
            tc.tile_pool(name="wpool", bufs=8) as wpool,
            tc.tile_pool(name="fstage", bufs=4) as fstage,
            tc.tile_pool(name="stats", bufs=4) as stats,
            tc.tile_pool(name="psum", bufs=2, space="PSUM") as psum,
        ):
            def pacc():
                return psum.tile([P, AW], F32, tag="pacc", bufs=PB, name="pacc")

            # ---- constants ----
            trimask = persist.tile([P, P], F32, tag="trimask", name="trimask")
            nc.sync.dma_start(out=trimask, in_=tri_d[:, :])

            # ---- persistent activations ----
            XT8 = persist.tile([P, DC, T], FP8E4, tag="XT8", name="XT8")
            V = persist.tile([P, TC, D], FP8E5, tag="V", name="V")  # V [j, v]
            Wv8 = persist.tile([P, DC, D], FP8E4, tag="Wv8", name="Wv8")
            # probs^T rows in pair-tiles for DoubleRow: pair p holds rows
            # jc=2p (at [:, 0, 0:]) and jc=2p+1 (at [:, 1, 128:]), both
            # covering i in [256*p, T).
            PT = [
                persist.tile(
                    [P, 2, T - 2 * P * p], FP8E5, tag=f"PT{p}", name=f"PT{p}"
                )
                for p in range(NP)
            ]
            # row 2p+1's first 128 columns are never written by exp but are
            # read by the pair matmuls -> must be zero.
            for p in range(NP):
                nc.gpsimd.memset(PT[p][:, 1, 0:P], 0.0)

            if with_bias:
                XT = persist.tile([P, DC, T], BF16, tag="XT", name="XT")
                ones = persist.tile([1, NS], BF16, tag="ones", name="ones")
                nc.vector.memset(ones, 1.0)
                b_sb = {}
                for nm, bd in (("q", bq_d), ("k", bk_d), ("v", bv_d)):
                    bt = persist.tile(
                        [1, D], BF16, tag=f"bias_{nm}", name=f"bias_{nm}"
                    )
                    nc.gpsimd.dma_start(out=bt, in_=bd[None, :])  # f32 -> bf16
                    b_sb[nm] = bt
                nc.vector.tensor_scalar(
                    out=b_sb["v"],
                    in0=b_sb["v"],
                    scalar1=WV_PRESCALE,
                    scalar2=None,
                    op0=mybir.AluOpType.mult,
                )
                QT = persist.tile([P, DC, T], BF16, tag="QT", name="QT")
                KT = persist.tile([P, DC, T], BF16, tag="KT", name="KT")
                ident = persist.tile([P, P], BF16, tag="ident", name="ident")
                from concourse.masks import make_identity

                make_identity(nc, ident)

                for ic in range(TC):  # X^T via PE transpose
                    xf = fstage.tile(
                        [P, D], F32, tag="f32stage", bufs=4, name="xf"
                    )
                    nc.sync.dma_start(out=xf, in_=x_d[ts(ic, P), :])
                    xb = fstage.tile([P, D], BF16, tag="xbf", bufs=2, name="xb")
                    nc.vector.tensor_copy(out=xb, in_=xf)
                    pt_ = psum.tile([P, D], BF16, tag="tacc", bufs=2, name="pt_")
                    for dc in range(DC):
                        nc.tensor.transpose(
                            pt_[:, ts(dc, P)], xb[:, ts(dc, P)], ident
                        )
                    nc.scalar.copy(
                        out=XT[:, :, ts(ic, P)],
                        in_=pt_.rearrange("p (dc c) -> p dc c", c=P),
                    )
                    nc.vector.tensor_copy(
                        out=XT8[:, :, ts(ic, P)], in_=XT[:, :, ts(ic, P)]
                    )

                def load_w_chunks(w_dram):
                    wt = []
                    for dc in range(DC):
                        w1 = wpool.tile([P, D], BF16, tag="w", bufs=8, name="w1")
                        nc.gpsimd.dma_start(out=w1, in_=w_dram[ts(dc, P), :])
                        wt.append(w1)
                    return wt

                for w_dram, bkey, dst in ((wq_d, "q", QT), (wk_d, "k", KT)):
                    wt = load_w_chunks(w_dram)
                    for m in range(DC):
                        for h in range(2):
                            acc = pacc()
                            for dc in range(DC):
                                for s2 in range(2):
                                    nc.tensor.matmul(
                                        acc[:, ts(s2, NS)],
                                        lhsT=wt[dc][:, ts(m, P)],
                                        rhs=XT[:, dc, ts(2 * h + s2, NS)],
                                        start=(dc == 0),
                                        stop=False,
                                    )
                            for s2 in range(2):
                                nc.tensor.matmul(
                                    acc[:, ts(s2, NS)],
                                    lhsT=b_sb[bkey][:, ts(m, P)],
                                    rhs=ones[:, :],
                                    start=False,
                                    stop=True,
                                )
                            nc.scalar.copy(
                                out=dst[:, m, ts(h, AW)], in_=acc
                            )

                wt = load_w_chunks(wv_d)
                for dc in range(DC):
                    nc.vector.tensor_scalar(
                        out=Wv8[:, dc, :],
                        in0=wt[dc][:, :],
                        scalar1=WV_PRESCALE,
                        scalar2=None,
                        op0=mybir.AluOpType.mult,
                    )
            else:
                # everything arrives compute-ready from the host; order the
                # loads so the first V matmuls (need Wv8/XT8 chunks 0-1) can
                # start as early as possible.
                M8 = persist.tile([P, DC, D], FP8E4, tag="M8", name="M8")
                XR = persist.tile([P, TC, D], F32, tag="XR", name="XR")
                for dc in range(2):
                    nc.sync.dma_start(
                        out=XT8[:, dc, 0:P], in_=xt8_d[ts(dc, P), 0:P]
                    )
                    nc.sync.dma_start(
                        out=Wv8[:, dc, :], in_=wv8_d[ts(dc, P), :]
                    )
                for dc in range(2):
                    nc.sync.dma_start(
                        out=XT8[:, dc, P:T], in_=xt8_d[ts(dc, P), P:T]
                    )
                for dc in range(2, DC):
                    nc.sync.dma_start(out=Wv8[:, dc, :], in_=wv8_d[ts(dc, P), :])
                    nc.sync.dma_start(out=XT8[:, dc, :], in_=xt8_d[ts(dc, P), :])
                for dc in range(DC):
                    nc.sync.dma_start(out=M8[:, dc, :], in_=m8_d[ts(dc, P), :])
                # residual prefetch (needed from the first E phase on)
                for ic in range(TC):
                    nc.sync.dma_start(out=XR[:, ic, :], in_=x_d[ts(ic, P), :])

            # V (fp8 DoubleRow): acc[j, v] = 32 * sum_d X[j, d] Wv[d, v]
            for jc in range(TC):
                acc = pacc()
                for dp in range(DC // 2):
                    for vs in range(2):
                        nc.tensor.matmul(
                            acc[:, ts(vs, NS)],
                            lhsT=XT8[:, 2 * dp : 2 * dp + 2, ts(jc, P)],
                            rhs=Wv8[:, 2 * dp : 2 * dp + 2, ts(vs, NS)],
                            start=(dp == 0),
                            stop=(not with_bias and dp == DC // 2 - 1),
                            perf_mode=DR,
                        )
                if with_bias:
                    for vs in range(2):
                        nc.tensor.matmul(
                            acc[:, ts(vs, NS)],
                            lhsT=ones[:, 0:P],
                            rhs=b_sb["v"][:, ts(vs, NS)],
                            start=False,
                            stop=True,
                        )
                nc.vector.tensor_copy(out=V[:, jc, :], in_=acc)

            if with_bias:
                def c_mms(acc, jc, mm_slices, base_i):
                    for kc in range(DC):
                        lt = KT[:, kc, ts(jc, P)]
                        for lo, hi in mm_slices:
                            nc.tensor.matmul(
                                acc[:, lo:hi],
                                lhsT=lt,
                                rhs=QT[:, kc, base_i + lo : base_i + hi],
                                start=(kc == 0),
                                stop=(kc == DC - 1),
                            )

                exp_scale = 1.0
            else:
                # Y^T = M^T X^T (fp8 DoubleRow): YT[e, i] = sum_d M[d,e] XT[d,i]
                # (carries the x32 M prescale; undone inside the softmax exp)
                Y8 = persist.tile([P, DC, T], FP8E4, tag="Y8", name="Y8")
                for m in range(DC):
                    for h in range(2):
                        acc = pacc()
                        for dp in range(DC // 2):
                            for s2 in range(2):
                                nc.tensor.matmul(
                                    acc[:, ts(s2, NS)],
                                    lhsT=M8[:, 2 * dp : 2 * dp + 2, ts(m, P)],
                                    rhs=XT8[
                                        :, 2 * dp : 2 * dp + 2, ts(2 * h + s2, NS)
                                    ],
                                    start=(dp == 0),
                                    stop=(dp == DC // 2 - 1),
                                    perf_mode=DR,
                                )
                        nc.scalar.copy(out=Y8[:, m, ts(h, AW)], in_=acc)

                def c_mms(acc, jc, mm_slices, base_i):
                    # LT[j, i] = sum_e X[j, e] Y[i, e]  (both e4m3, DoubleRow)
                    for kp in range(DC // 2):
                        lt = XT8[:, 2 * kp : 2 * kp + 2, ts(jc, P)]
                        for lo, hi in mm_slices:
                            nc.tensor.matmul(
                                acc[:, lo:hi],
                                lhsT=lt,
                                rhs=Y8[
                                    :, 2 * kp : 2 * kp + 2, base_i + lo : base_i + hi
                                ],
                                start=(kp == 0),
                                stop=(kp == DC // 2 - 1),
                                perf_mode=DR,
                            )

                exp_scale = 1.0 / M_SCALE

            # ====== phases C+E interleaved ======
            # C_jc: LT row-chunk jc (i >= 128*jc) in 1-2 [128,1024] sub-accs
            #       + split softmax (sub reduce_max -> combined max -> per-sub
            #       exp with accumulated sums)
            # E_ic: read[ic] = probs @ V + residual + store
            def phase_c(jc):
                g, r = jc // 4, jc % 4
                off = NS * g
                dstart = P * r
                width = T - off  # valid acc cols [dstart, width)
                pr, rr = jc // 2, jc % 2
                n_sub = 2 if width > AW else 1
                subs = []
                nmx = []
                for u in range(n_sub):
                    acc = pacc()
                    c0 = dstart if u == 0 else 0
                    hi = min(width - AW * u, AW)
                    base_i = off + AW * u
                    mm_slices = []
                    lo = c0
                    while lo < hi:
                        sl_end = min((lo // NS + 1) * NS, hi)
                        mm_slices.append((lo, sl_end))
                        lo = sl_end
                    c_mms(acc, jc, mm_slices, base_i)
                    if u == 0:
                        nc.vector.tensor_add(
                            out=acc[:, dstart : dstart + P],
                            in0=acc[:, dstart : dstart + P],
                            in1=trimask,
                        )
                    nm = stats.tile(
                        [P, 1], F32, tag="negmax", bufs=8, name="negmax"
                    )
                    nc.vector.reduce_max(
                        out=nm,
                        in_=acc[:, c0:hi],
                        axis=mybir.AxisListType.X,
                        negate=True,
                    )
                    nmx.append(nm)
                    subs.append((acc, c0, hi, base_i))
                if n_sub == 2:
                    nmc = stats.tile([P, 1], F32, tag="nmc", bufs=4, name="nmc")
                    nc.vector.tensor_tensor(
                        out=nmc, in0=nmx[0], in1=nmx[1], op=mybir.AluOpType.min
                    )
                else:
                    nmc = nmx[0]
                if exp_scale != 1.0:
                    nm2 = stats.tile([P, 1], F32, tag="nm2", bufs=4, name="nm2")
                    nc.vector.tensor_scalar(
                        out=nm2,
                        in0=nmc,
                        scalar1=exp_scale,
                        scalar2=None,
                        op0=mybir.AluOpType.mult,
                    )
                    ebias = nm2
                else:
                    ebias = nmc
                ssums = []
                for acc, c0, hi, base_i in subs:
                    ss = stats.tile([P, 1], F32, tag="ssum", bufs=8, name="ssum")
                    pt_lo = base_i + c0 - 2 * P * pr
                    nc.scalar.activation(
                        out=PT[pr][:, rr, pt_lo : pt_lo + (hi - c0)],
                        in_=acc[:, c0:hi],
                        func=mybir.ActivationFunctionType.Exp,
                        bias=ebias,
                        scale=exp_scale,
                        accum_out=ss,
                    )
                    ssums.append(ss)
                if len(ssums) == 2:
                    sst = stats.tile([P, 1], F32, tag="sst", bufs=4, name="sst")
                    nc.vector.tensor_add(out=sst, in0=ssums[0], in1=ssums[1])
                else:
                    sst = ssums[0]
                rv = stats.tile([P, 1], F32, tag="rv", bufs=4, name="rv")
                nc.vector.reciprocal(out=rv, in_=sst)
                # fold softmax denominator, 1/sqrt(1024) and the 1/32 weight
                # pre-scale compensation into V's rows: V[j, :] *= rv[j]/1024
                nc.vector.tensor_scalar(
                    out=V[:, jc, :],
                    in0=V[:, jc, :],
                    scalar1=rv,
                    scalar2=1.0 / (32.0 * WV_PRESCALE),
                    op0=mybir.AluOpType.mult,
                    op1=mybir.AluOpType.mult,
                )

            def phase_e(ic):
                acc = pacc()
                np_ic = ic // 2 + 1  # pairs 0..ic//2
                for p in range(np_ic):
                    blk = PT[p][
                        :, :, ic * P - 2 * P * p : (ic + 1) * P - 2 * P * p
                    ]
                    for vs in range(2):
                        nc.tensor.matmul(
                            acc[:, ts(vs, NS)],
                            lhsT=blk,
                            rhs=V[:, 2 * p : 2 * p + 2, ts(vs, NS)],
                            start=(p == 0),
                            stop=(p == np_ic - 1),
                            perf_mode=DR,
                        )
                ot = fstage.tile([P, D], F32, tag="f32stage", bufs=4, name="ot")
                if with_bias:
                    xf = fstage.tile(
                        [P, D], F32, tag="f32stage", bufs=4, name="xf"
                    )
                    nc.sync.dma_start(out=xf, in_=x_d[ts(ic, P), :])
                    nc.vector.tensor_add(out=ot, in0=acc, in1=xf)
                else:
                    nc.vector.tensor_add(out=ot, in0=acc, in1=XR[:, ic, :])
                nc.sync.dma_start(out=out_d[ts(ic, P), :], in_=ot)

            # E_{t-2} emitted before C_t: E fills the PE while C_{t-1}'s
            # softmax drains; E_{t-2}'s last pair needs V row t-1 scaled
            # (end of C_{t-1}'s chain), which is ready by then.
            phase_c(0)
            phase_c(1)
            for jc in range(2, TC):
                phase_e(jc - 2)
                phase_c(jc)
            phase_e(TC - 2)
            phase_e(TC - 1)

    nc.finalize()
    return nc


_NC_CACHE = {}


def get_nc(with_bias: bool = False):
    if with_bias not in _NC_CACHE:
        _NC_CACHE[with_bias] = build_nc(with_bias)
    return _NC_CACHE[with_bias]


def make_in_maps(inputs: dict, with_bias: bool | None = None) -> list[dict]:
    if with_bias is None:
        with_bias = needs_bias(inputs)
    mb = np.ascontiguousarray(np.asarray(inputs["minibatch"], dtype=np.float32))
    assert mb.shape == (B, T, D)
    shared = {"tri_mask": host_tri_mask()}
    if with_bias:
        for k in ("Wq", "bq", "Wk", "bk", "Wv", "bv"):
            shared[k] = np.ascontiguousarray(
                np.asarray(inputs[k], dtype=np.float32)
            )
        return [{"minibatch": mb[c], **shared} for c in range(N_CORES)]
    # alternate layouts/dtypes of the same inputs -> no device transposes
    # or dtype-conversion passes; M = Wq Wk^T is weights-only preprocessing
    wq = np.asarray(inputs["Wq"], dtype=np.float32)
    wk = np.asarray(inputs["Wk"], dtype=np.float32)
    wv = np.asarray(inputs["Wv"], dtype=np.float32)
    shared["M_fp8"] = np.ascontiguousarray((wq @ wk.T) * M_SCALE).astype(
        NP_FP8E4
    )
    shared["Wv_fp8"] = np.ascontiguousarray(wv * WV_PRESCALE).astype(NP_FP8E4)
    maps = []
    for c in range(N_CORES):
        xt = np.ascontiguousarray(mb[c].T)
        maps.append(
            {
                "minibatch": mb[c],
                "xt_fp8": xt.astype(NP_FP8E4),
                **shared,
            }
        )
    return maps


def needs_bias(inputs: dict) -> bool:
    return any(
        np.any(np.asarray(inputs[k], dtype=np.float32) != 0.0)
        for k in ("bq", "bk", "bv")
    )


def kernel(**inputs) -> np.ndarray:
    wb = needs_bias(inputs)
    nc = get_nc(with_bias=wb)
    in_maps = make_in_maps(inputs, with_bias=wb)
    res = run_bass_kernel_spmd(nc, in_maps, core_ids=list(range(N_CORES)))
    return np.stack([res.results[c]["out"] for c in range(N_CORES)], axis=0)


if __name__ == "__main__":
    rng = np.random.default_rng(0)
    demo = {
        "minibatch": rng.standard_normal((B, T, D), dtype=np.float32),
        "Wq": rng.standard_normal((D, D), dtype=np.float32) * 0.02,
        "bq": np.zeros(D, np.float32),
        "Wk": rng.standard_normal((D, D), dtype=np.float32) * 0.02,
        "bk": np.zeros(D, np.float32),
        "Wv": rng.standard_normal((D, D), dtype=np.float32) * 0.02,
        "bv": np.zeros(D, np.float32),
    }
    out = kernel(**demo)
    print(out.shape, out.dtype)


# revision 11
# speedup vs baseline: 1.1518x; 1.0427x over previous
"""Trainium2 Bass kernel: causal attention block with query-axis softmax.

Reference math (per batch element b):
    Q = X @ Wq + bq ; K = X @ Wk + bk ; V = X @ Wv + bv          # [T, D]
    logits[i, j] = Q[i] . K[j],  logits[i, j] = -inf where j > i
    probs = softmax(logits, axis=i) / sqrt(1024)                 # QUERY axis
    out = X + probs @ V
Distribution: pure data-parallel — B=8 batch elements, one per NeuronCore,
weights replicated, no collectives.

Per-core implementation notes (zero-bias fast path, all-fp8 PE):
  * Works in "transposed logit" space LT[j, i] = logits[i, j], so the
    axis-i softmax is a per-partition free-axis reduction.
  * logits = X (Wq Wk^T) X^T: M = Wq Wk^T is computed once on the HOST
    (weights-only preprocessing, like the layout/dtype preprocessing),
    shipped as e4m3 with a x32 prescale to clear e4m3's subnormal range.
  * Every PE pass runs fp8 DoubleRow (157 TF/s, 2 contraction rows per
    pass): Y^T = M^T X^T from e4m3; LT = X Y^T from e4m3 (Y re-quantized
    to e4m3 on device); V = X@Wv from e4m3; probs@V from e5m2.
  * The x32 M-prescale is undone inside the softmax exp (activation
    scale=1/32); the softmax denominator, the 1/sqrt(1024) scale and the
    1/32 Wv-prescale compensation are folded into V's rows.
  * All PSUM accumulators are [128, 1024] (2 banks), one pool tag with 4
    rotating buffers.  Wide logit rows are split into two sub-accs with
    split reduce_max / exp (combined via a [128,1] min), so a C phase
    releases PSUM banks as soon as each half's exp has read them — the
    next C's matmuls never wait on a full softmax chain.
  * E phases are emitted BEFORE the next C phase so their matmuls fill
    the PE while the previous C row's softmax drains.
  * Residual X chunks are prefetched into SBUF during the projection
    phases; fp32 residual add + store.
  * Causal structure: LT row-chunk jc computes only i >= 128*jc; probs
    rows live in pair-tiles so the DoubleRow probs@V matmuls skip
    fully-masked pairs.

The with_bias=True fallback (never taken for this problem's all-zero
biases) keeps the direct bf16 Q/K projection structure with PE
transposes.
"""

import sys

if "/opt/trn_rl_repo" not in sys.path:
    sys.path.insert(0, "/opt/trn_rl_repo")

import numpy as np

import concourse.bass as bass
import concourse.mybir as mybir
import concourse.tile as tile
from concourse import bacc
from concourse.bass import ts
from concourse.bass_utils import run_bass_kernel_spmd

B, T, D = 8, 2048, 1024
P = 128
DC = D // P  # 8 feature chunks
TC = T // P  # 16 token chunks
NP = TC // 2  # 8 token-chunk pairs (DoubleRow)
NS = 512  # matmul moving free-dim
SL = T // NS  # 4 slices per full row
AW = 1024  # PSUM accumulator width (2 banks)
F32 = mybir.dt.float32
BF16 = mybir.dt.bfloat16
FP8E4 = mybir.dt.float8e4  # e4m3
FP8E5 = mybir.dt.float8e5  # e5m2
NEG = -1.0e30
N_CORES = 8
WV_PRESCALE = 32.0  # keeps 32*Wv in e4m3's normal range (|Wv| ~ 0.02)
M_SCALE = 32.0  # keeps 32*(Wq Wk^T) in e4m3's normal range

NP_BF16 = mybir.dt.np(BF16)
NP_FP8E4 = mybir.dt.np(FP8E4)

DR = mybir.MatmulPerfMode.DoubleRow


def host_tri_mask() -> np.ndarray:
    """[128, 128] additive mask for the diagonal block of LT row-chunk jc:
    entry [p, c] (j = jc*128+p, i = jc*128+c) is 0 where i >= j else -1e30."""
    p = np.arange(P)[:, None]
    c = np.arange(P)[None, :]
    return np.where(c >= p, 0.0, NEG).astype(np.float32)


def build_nc(with_bias: bool):
    nc = bacc.Bacc("TRN2", target_bir_lowering=False, debug=False)

    x_d = nc.declare_dram_parameter("minibatch", [T, D], F32, isOutput=False)
    tri_d = nc.declare_dram_parameter("tri_mask", [P, P], F32, isOutput=False)
    out_d = nc.declare_dram_parameter("out", [T, D], F32, isOutput=True)
    if with_bias:
        wq_d = nc.declare_dram_parameter("Wq", [D, D], F32, isOutput=False)
        bq_d = nc.declare_dram_parameter("bq", [D], F32, isOutput=False)
        wk_d = nc.declare_dram_parameter("Wk", [D, D], F32, isOutput=False)
        bk_d = nc.declare_dram_parameter("bk", [D], F32, isOutput=False)
        wv_d = nc.declare_dram_parameter("Wv", [D, D], F32, isOutput=False)
        bv_d = nc.declare_dram_parameter("bv", [D], F32, isOutput=False)
    else:
        xt8_d = nc.declare_dram_parameter("xt_fp8", [D, T], FP8E4, isOutput=False)
        m8_d = nc.declare_dram_parameter("M_fp8", [D, D], FP8E4, isOutput=False)
        wv8_d = nc.declare_dram_parameter("Wv_fp8", [D, D], FP8E4, isOutput=False)

    PB = 3 if with_bias else 4  # pacc rotation depth (bias path needs a bank
    # for its transpose staging tile)

    with tile.TileContext(nc) as tc:
        with (
            tc.tile_pool(name="persist", bufs=1) as persist,
            tc.tile_pool(name="wpool", bufs=8) as wpool,
            tc.tile_pool(name="fstage", bufs=4) as fstage,
            tc.tile_pool(name="stats", bufs=4) as stats,
            tc.tile_pool(name="psum", bufs=2, space="PSUM") as psum,
        ):
            def pacc():
                return psum.tile([P, AW], F32, tag="pacc", bufs=PB, name="pacc")

            # ---- constants ----
            trimask = persist.tile([P, P], F32, tag="trimask", name="trimask")
            nc.sync.dma_start(out=trimask, in_=tri_d[:, :])

            # ---- persistent activations ----
            XT8 = persist.tile([P, DC, T], FP8E4, tag="XT8", name="XT8")
            V = persist.tile([P, TC, D], FP8E5, tag="V", name="V")  # V [j, v]
            Wv8 = persist.tile([P, DC, D], FP8E4, tag="Wv8", name="Wv8")
            # probs^T rows in pair-tiles for DoubleRow: pair p holds rows
            # jc=2p (at [:, 0, 0:]) and jc=2p+1 (at [:, 1, 128:]), both
            # covering i in [256*p, T).
            PT = [
                persist.tile(
                    [P, 2, T - 2 * P * p], FP8E5, tag=f"PT{p}", name=f"PT{p}"
                )
                for p in range(NP)
            ]
            # row 2p+1's first 128 columns are never written by exp but are
            # read by the pair matmuls -> must be zero.
            for p in range(NP):
                nc.gpsimd.memset(PT[p][:, 1, 0:P], 0.0)

            if with_bias:
                XT = persist.tile([P, DC, T], BF16, tag="XT", name="XT")
                ones = persist.tile([1, NS], BF16, tag="ones", name="ones")
                nc.vector.memset(ones, 1.0)
                b_sb = {}
                for nm, bd in (("q", bq_d), ("k", bk_d), ("v", bv_d)):
                    bt = persist.tile(
                        [1, D], BF16, tag=f"bias_{nm}", name=f"bias_{nm}"
                    )
                    nc.gpsimd.dma_start(out=bt, in_=bd[None, :])  # f32 -> bf16
                    b_sb[nm] = bt
                nc.vector.tensor_scalar(
                    out=b_sb["v"],
                    in0=b_sb["v"],
                    scalar1=WV_PRESCALE,
                    scalar2=None,
                    op0=mybir.AluOpType.mult,
                )
                QT = persist.tile([P, DC, T], BF16, tag="QT", name="QT")
                KT = persist.tile([P, DC, T], BF16, tag="KT", name="KT")
                ident = persist.tile([P, P], BF16, tag="ident", name="ident")
                from concourse.masks import make_identity

                make_identity(nc, ident)

                for ic in range(TC):  # X^T via PE transpose
                    xf = fstage.tile(
                        [P, D], F32, tag="f32stage", bufs=4, name="xf"
                    )
                    nc.sync.dma_start(out=xf, in_=x_d[ts(ic, P), :])
                    xb = fstage.tile([P, D], BF16, tag="xbf", bufs=2, name="xb")
                    nc.vector.tensor_copy(out=xb, in_=xf)
                    pt_ = psum.tile([P, D], BF16, tag="tacc", bufs=2, name="pt_")
                    for dc in range(DC):
                        nc.tensor.transpose(
                            pt_[:, ts(dc, P)], xb[:, ts(dc, P)], ident
                        )
                    nc.scalar.copy(
                        out=XT[:, :, ts(ic, P)],
                        in_=pt_.rearrange("p (dc c) -> p dc c", c=P),
                    )
                    nc.vector.tensor_copy(
                        out=XT8[:, :, ts(ic, P)], in_=XT[:, :, ts(ic, P)]
                    )

                def load_w_chunks(w_dram):
                    wt = []
                    for dc in range(DC):
                        w1 = wpool.tile([P, D], BF16, tag="w", bufs=8, name="w1")
                        nc.gpsimd.dma_start(out=w1, in_=w_dram[ts(dc, P), :])
                        wt.append(w1)
                    return wt

                for w_dram, bkey, dst in ((wq_d, "q", QT), (wk_d, "k", KT)):
                    wt = load_w_chunks(w_dram)
                    for m in range(DC):
                        for h in range(2):
                            acc = pacc()
                            for dc in range(DC):
                                for s2 in range(2):
                                    nc.tensor.matmul(
                                        acc[:, ts(s2, NS)],
                                        lhsT=wt[dc][:, ts(m, P)],
                                        rhs=XT[:, dc, ts(2 * h + s2, NS)],
                                        start=(dc == 0),
                                        stop=False,
                                    )
                            for s2 in range(2):
                                nc.tensor.matmul(
                                    acc[:, ts(s2, NS)],
                                    lhsT=b_sb[bkey][:, ts(m, P)],
                                    rhs=ones[:, :],
                                    start=False,
                                    stop=True,
                                )
                            nc.scalar.copy(
                                out=dst[:, m, ts(h, AW)], in_=acc
                            )

                wt = load_w_chunks(wv_d)
                for dc in range(DC):
                    nc.vector.tensor_scalar(
                        out=Wv8[:, dc, :],
                        in0=wt[dc][:, :],
                        scalar1=WV_PRESCALE,
                        scalar2=None,
                        op0=mybir.AluOpType.mult,
                    )
            else:
                # everything arrives compute-ready from the host; order the
                # loads so the first V matmuls (need Wv8/XT8 chunks 0-1) can
                # start as early as possible.
                M8 = persist.tile([P, DC, D], FP8E4, tag="M8", name="M8")
                XR = persist.tile([P, TC, D], F32, tag="XR", name="XR")
                for dc in range(2):
                    nc.sync.dma_start(
                        out=XT8[:, dc, 0:P], in_=xt8_d[ts(dc, P), 0:P]
                    )
                    nc.sync.dma_start(
                        out=Wv8[:, dc, :], in_=wv8_d[ts(dc, P), :]
                    )
                for dc in range(2):
                    nc.sync.dma_start(
                        out=XT8[:, dc, P:T], in_=xt8_d[ts(dc, P), P:T]
                    )
                for dc in range(2, DC):
                    nc.sync.dma_start(out=Wv8[:, dc, :], in_=wv8_d[ts(dc, P), :])
                    nc.sync.dma_start(out=XT8[:, dc, :], in_=xt8_d[ts(dc, P), :])
                for dc in range(DC):
                    nc.sync.dma_start(out=M8[:, dc, :], in_=m8_d[ts(dc, P), :])
                # residual prefetch (needed from the first E phase on)
                for ic in range(TC):
                    nc.sync.dma_start(out=XR[:, ic, :], in_=x_d[ts(ic, P), :])

            # V (fp8 DoubleRow): acc[j, v] = 32 * sum_d X[j, d] Wv[d, v]
            for jc in range(TC):
                acc = pacc()
                for dp in range(DC // 2):
                    for vs in range(2):
                        nc.tensor.matmul(
                            acc[:, ts(vs, NS)],
                            lhsT=XT8[:, 2 * dp : 2 * dp + 2, ts(jc, P)],
                            rhs=Wv8[:, 2 * dp : 2 * dp + 2, ts(vs, NS)],
                            start=(dp == 0),
                            stop=(not with_bias and dp == DC // 2 - 1),
                            perf_mode=DR,
                        )
                if with_bias:
                    for vs in range(2):
                        nc.tensor.matmul(
                            acc[:, ts(vs, NS)],
                            lhsT=ones[:, 0:P],
                            rhs=b_sb["v"][:, ts(vs, NS)],
                            start=False,
                            stop=True,
                        )
                nc.vector.tensor_copy(out=V[:, jc, :], in_=acc)

            if with_bias:
                def c_mms(acc, jc, mm_slices, base_i):
                    for kc in range(DC):
                        lt = KT[:, kc, ts(jc, P)]
                        for lo, hi in mm_slices:
                            nc.tensor.matmul(
                                acc[:, lo:hi],
                                lhsT=lt,
                                rhs=QT[:, kc, base_i + lo : base_i + hi],
                                start=(kc == 0),
                                stop=(kc == DC - 1),
                            )

                exp_scale = 1.0
            else:
                # Y^T = M^T X^T (fp8 DoubleRow): YT[e, i] = sum_d M[d,e] XT[d,i]
                # (carries the x32 M prescale; undone inside the softmax exp)
                Y8 = persist.tile([P, DC, T], FP8E4, tag="Y8", name="Y8")
                for m in range(DC):
                    for h in range(2):
                        acc = pacc()
                        for dp in range(DC // 2):
                            for s2 in range(2):
                                nc.tensor.matmul(
                                    acc[:, ts(s2, NS)],
                                    lhsT=M8[:, 2 * dp : 2 * dp + 2, ts(m, P)],
                                    rhs=XT8[
                                        :, 2 * dp : 2 * dp + 2, ts(2 * h + s2, NS)
                                    ],
                                    start=(dp == 0),
                                    stop=(dp == DC // 2 - 1),
                                    perf_mode=DR,
                                )
                        nc.scalar.copy(out=Y8[:, m, ts(h, AW)], in_=acc)

                def c_mms(acc, jc, mm_slices, base_i):
                    # LT[j, i] = sum_e X[j, e] Y[i, e]  (both e4m3, DoubleRow)
                    for kp in range(DC // 2):
                        lt = XT8[:, 2 * kp : 2 * kp + 2, ts(jc, P)]
                        for lo, hi in mm_slices:
                            nc.tensor.matmul(
                                acc[:, lo:hi],
                                lhsT=lt,
                                rhs=Y8[
                                    :, 2 * kp : 2 * kp + 2, base_i + lo : base_i + hi
                                ],
                                start=(kp == 0),
                                stop=(kp == DC // 2 - 1),
                                perf_mode=DR,
                            )

                exp_scale = 1.0 / M_SCALE

            # ====== phases C+E interleaved ======
            # C_jc: LT row-chunk jc (i >= 128*jc) in 1-2 [128,1024] sub-accs
            #       + split softmax (sub reduce_max -> combined max -> per-sub
            #       exp with accumulated sums)
            # E_ic: read[ic] = probs @ V + residual + store
            def phase_c(jc):
                g, r = jc // 4, jc % 4
                off = NS * g
                dstart = P * r
                width = T - off  # valid acc cols [dstart, width)
                pr, rr = jc // 2, jc % 2
                n_sub = 2 if width > AW else 1
                subs = []
                nmx = []
                for u in range(n_sub):
                    acc = pacc()
                    c0 = dstart if u == 0 else 0
                    hi = min(width - AW * u, AW)
                    base_i = off + AW * u
                    mm_slices = []
                    lo = c0
                    while lo < hi:
                        sl_end = min((lo // NS + 1) * NS, hi)
                        mm_slices.append((lo, sl_end))
                        lo = sl_end
                    c_mms(acc, jc, mm_slices, base_i)
                    if u == 0:
                        nc.vector.tensor_add(
                            out=acc[:, dstart : dstart + P],
                            in0=acc[:, dstart : dstart + P],
                            in1=trimask,
                        )
                    nm = stats.tile(
                        [P, 1], F32, tag="negmax", bufs=8, name="negmax"
                    )
                    nc.vector.reduce_max(
                        out=nm,
                        in_=acc[:, c0:hi],
                        axis=mybir.AxisListType.X,
                        negate=True,
                    )
                    nmx.append(nm)
                    subs.append((acc, c0, hi, base_i))
                if n_sub == 2:
                    nmc = stats.tile([P, 1], F32, tag="nmc", bufs=4, name="nmc")
                    nc.vector.tensor_tensor(
                        out=nmc, in0=nmx[0], in1=nmx[1], op=mybir.AluOpType.min
                    )
                else:
                    nmc = nmx[0]
                if exp_scale != 1.0:
                    # on the scalar engine: feeds the exps right after, and
                    # keeps the [P,1] op off the busy vector queue
                    nm2 = stats.tile([P, 1], F32, tag="nm2", bufs=4, name="nm2")
                    nc.scalar.activation(
                        out=nm2,
                        in_=nmc,
                        func=mybir.ActivationFunctionType.Copy,
                        scale=exp_scale,
                    )
                    ebias = nm2
                else:
                    ebias = nmc
                ssums = []
                for acc, c0, hi, base_i in subs:
                    ss = stats.tile([P, 1], F32, tag="ssum", bufs=8, name="ssum")
                    pt_lo = base_i + c0 - 2 * P * pr
                    nc.scalar.activation(
                        out=PT[pr][:, rr, pt_lo : pt_lo + (hi - c0)],
                        in_=acc[:, c0:hi],
                        func=mybir.ActivationFunctionType.Exp,
                        bias=ebias,
                        scale=exp_scale,
                        accum_out=ss,
                    )
                    ssums.append(ss)
                if len(ssums) == 2:
                    sst = stats.tile([P, 1], F32, tag="sst", bufs=4, name="sst")
                    nc.vector.tensor_add(out=sst, in0=ssums[0], in1=ssums[1])
                else:
                    sst = ssums[0]
                # rv = 1/(1024*S): folds the softmax denominator, 1/sqrt(1024)
                # and the 1/32 weight pre-scale compensation
                rv = stats.tile([P, 1], F32, tag="rv", bufs=4, name="rv")
                nc.vector.reciprocal(out=rv, in_=sst)
                rv2 = stats.tile([P, 1], F32, tag="rv2", bufs=4, name="rv2")
                nc.vector.tensor_scalar(
                    out=rv2,
                    in0=rv,
                    scalar1=1.0 / (32.0 * WV_PRESCALE),
                    scalar2=None,
                    op0=mybir.AluOpType.mult,
                )
                # V[j, :] *= rv2[j] on the scalar engine (Copy with per-
                # partition scale) — keeps the [P,1024] pass off the busy
                # vector queue
                nc.scalar.activation(
                    out=V[:, jc, :],
                    in_=V[:, jc, :],
                    func=mybir.ActivationFunctionType.Copy,
                    scale=rv2,
                )

            def phase_e(ic):
                acc = pacc()
                np_ic = ic // 2 + 1  # pairs 0..ic//2
                for p in range(np_ic):
                    blk = PT[p][
                        :, :, ic * P - 2 * P * p : (ic + 1) * P - 2 * P * p
                    ]
                    for vs in range(2):
                        nc.tensor.matmul(
                            acc[:, ts(vs, NS)],
                            lhsT=blk,
                            rhs=V[:, 2 * p : 2 * p + 2, ts(vs, NS)],
                            start=(p == 0),
                            stop=(p == np_ic - 1),
                            perf_mode=DR,
                        )
                ot = fstage.tile([P, D], F32, tag="f32stage", bufs=4, name="ot")
                if with_bias:
                    xf = fstage.tile(
                        [P, D], F32, tag="f32stage", bufs=4, name="xf"
                    )
                    nc.sync.dma_start(out=xf, in_=x_d[ts(ic, P), :])
                    nc.vector.tensor_add(out=ot, in0=acc, in1=xf)
                else:
                    nc.vector.tensor_add(out=ot, in0=acc, in1=XR[:, ic, :])
                nc.sync.dma_start(out=out_d[ts(ic, P), :], in_=ot)

            # lag-4 interleave, E emitted before C: E_{t-4} fills the PE
            # while C_{t-1}'s softmax drains, and needs V row t-3 scaled
            # (end of C_{t-3}'s chain) — 3 phases of slack, so neither the
            # pipeline fill (C_0..C_3 first) nor steady state ever stalls
            # the PE on a softmax chain.
            for jc in range(4):
                phase_c(jc)
            for jc in range(4, TC):
                phase_e(jc - 4)
                phase_c(jc)
            for ic in range(TC - 4, TC):
                phase_e(ic)

    nc.finalize()
    return nc


_NC_CACHE = {}


def get_nc(with_bias: bool = False):
    if with_bias not in _NC_CACHE:
        _NC_CACHE[with_bias] = build_nc(with_bias)
    return _NC_CACHE[with_bias]


def make_in_maps(inputs: dict, with_bias: bool | None = None) -> list[dict]:
    if with_bias is None:
        with_bias = needs_bias(inputs)
    mb = np.ascontiguousarray(np.asarray(inputs["minibatch"], dtype=np.float32))
    assert mb.shape == (B, T, D)
    shared = {"tri_mask": host_tri_mask()}
    if with_bias:
        for k in ("Wq", "bq", "Wk", "bk", "Wv", "bv"):
            shared[k] = np.ascontiguousarray(
                np.asarray(inputs[k], dtype=np.float32)
            )
        return [{"minibatch": mb[c], **shared} for c in range(N_CORES)]
    # alternate layouts/dtypes of the same inputs -> no device transposes
    # or dtype-conversion passes; M = Wq Wk^T is weights-only preprocessing
    wq = np.asarray(inputs["Wq"], dtype=np.float32)
    wk = np.asarray(inputs["Wk"], dtype=np.float32)
    wv = np.asarray(inputs["Wv"], dtype=np.float32)
    shared["M_fp8"] = np.ascontiguousarray((wq @ wk.T) * M_SCALE).astype(
        NP_FP8E4
    )
    shared["Wv_fp8"] = np.ascontiguousarray(wv * WV_PRESCALE).astype(NP_FP8E4)
    maps = []
    for c in range(N_CORES):
        xt = np.ascontiguousarray(mb[c].T)
        maps.append(
            {
                "minibatch": mb[c],
                "xt_fp8": xt.astype(NP_FP8E4),
                **shared,
            }
        )
    return maps


def needs_bias(inputs: dict) -> bool:
    return any(
        np.any(np.asarray(inputs[k], dtype=np.float32) != 0.0)
        for k in ("bq", "bk", "bv")
    )


def kernel(**inputs) -> np.ndarray:
    wb = needs_bias(inputs)
    nc = get_nc(with_bias=wb)
    in_maps = make_in_maps(inputs, with_bias=wb)
    res = run_bass_kernel_spmd(nc, in_maps, core_ids=list(range(N_CORES)))
    return np.stack([res.results[c]["out"] for c in range(N_CORES)], axis=0)


if __name__ == "__main__":
    rng = np.random.default_rng(0)
    demo = {
        "minibatch": rng.standard_normal((B, T, D), dtype=np.float32),
        "Wq": rng.standard_normal((D, D), dtype=np.float32) * 0.02,
        "bq": np.zeros(D, np.float32),
        "Wk": rng.standard_normal((D, D), dtype=np.float32) * 0.02,
        "bk": np.zeros(D, np.float32),
        "Wv": rng.standard_normal((D, D), dtype=np.float32) * 0.02,
        "bv": np.zeros(D, np.float32),
    }
    out = kernel(**demo)
    print(out.shape, out.dtype)
